# revision 4
# baseline (speedup 1.0000x reference)
"""Multi-head attention (B=2, S=2048, D=1024, H=16, Dk=64) on 8 TRN2 cores.

Sharding: core c handles batch b=c//4 and head group g=c%4 (heads 4g..4g+3,
i.e. projection output dims 256g..256g+256). Fully independent cores, no
collectives.

Device pipeline per core (all matmul inputs bf16, fp32 PSUM accumulation):
  - K/Q projections into transposed layout  QT/KT [256 dims, 2048 seq]
    (lhsT = W^T chunk, rhs = x^T chunk; bias added via ACT Identity+bias
    during the PSUM->SBUF copy; Wq pre-scaled by 1/8 = 1/sqrt(Dk) on host).
  - V projection into natural layout VH [seq, dims], with host-augmented
    weights: per head 65 columns where col 64 is an all-ones output dim
    (gives the softmax denominator for free), and an extra ones-row in x^T
    + bias-row in W^T implements the bias add (K=1 extra accumulation).
  - Scores computed transposed: S^T[kv,q] = KT^T-block @ QT-block, two heads
    row-packed into one [128,1024] PSUM tile (2 banks). Causal mask added as
    bf16 -1e9 tiles (4 diagonal variants) on DVE; exp on ACT (no max
    subtraction needed: |scores| <= ~4); E in bf16.
  - PV: O'^T[65, q] += VH'-block^T-as-lhsT @ E-block, accumulated over kv
    blocks in PSUM. Row 64 = sum(E) = denominator.
  - O'^T copied to SBUF and DMA'd out; final division + head interleave on
    host.
"""

import numpy as np
import ml_dtypes

B, S, D, H, DK = 2, 2048, 1024, 16, 64
N_CORES = 8
HPC = 4          # heads per core
GD = HPC * DK    # group dims = 256
QC = 512         # q-chunk (also seq projection chunk)
KB = 128         # kv block
N_QC = S // QC   # 4
N_KB = S // KB   # 16
bf16 = ml_dtypes.bfloat16

_cache: dict = {}


def _build(mode: str):
    """mode: 'causal' (4 resident mask variants, upper blocks skipped),
    'none' (no masking, all blocks), 'general' (per-block masks from DRAM)."""
    import concourse.bass as bass
    import concourse.mybir as mybir
    from concourse import bacc
    from concourse.tile import TileContext

    fp32 = mybir.dt.float32
    bf = mybir.dt.bfloat16
    AF = mybir.ActivationFunctionType

    nc = bacc.Bacc("TRN2", target_bir_lowering=False, debug=False,
                   num_devices=N_CORES)

    xqT = nc.dram_tensor("xqT", [D, S], bf, kind="ExternalInput")
    xkT = nc.dram_tensor("xkT", [D, S], bf, kind="ExternalInput")
    xvT = nc.dram_tensor("xvT", [D + 1, S], bf, kind="ExternalInput")
    wqT = nc.dram_tensor("wqT", [D, GD], bf, kind="ExternalInput")
    wkT = nc.dram_tensor("wkT", [D, GD], bf, kind="ExternalInput")
    wvT = nc.dram_tensor("wvT", [D + 1, HPC * 65], bf, kind="ExternalInput")
    bqk = nc.dram_tensor("bqk", [128, 4], fp32, kind="ExternalInput")
    if mode == "causal":
        cmask = nc.dram_tensor("cmask", [4, 128, QC], bf, kind="ExternalInput")
    elif mode == "general":
        amaskT = nc.dram_tensor("amaskT", [S, S], bf, kind="ExternalInput")
    out = nc.dram_tensor("out", [HPC, 65, S], fp32, kind="ExternalOutput")

    NKC = D // 128  # 8 contraction chunks

    with TileContext(nc) as tc:
        with (
            tc.tile_pool(name="res", bufs=1) as res,
            tc.tile_pool(name="xin", bufs=16) as xin,
            tc.tile_pool(name="xrow", bufs=2) as xrow,
            tc.tile_pool(name="mload", bufs=4) as mload,
            tc.tile_pool(name="eload", bufs=4) as eload,
            tc.tile_pool(name="oout", bufs=4) as oout,
            tc.tile_pool(name="pproj", bufs=2, space="PSUM") as pproj,
            tc.tile_pool(name="pscore", bufs=2, space="PSUM") as pscore,
            tc.tile_pool(name="ppv", bufs=2, space="PSUM") as ppv,
        ):
            # ---- resident weight/bias/mask tiles ----
            wq_s = res.tile([128, NKC * GD], bf, tag="wq")
            wk_s = res.tile([128, NKC * GD], bf, tag="wk")
            wv_s = res.tile([128, NKC * HPC * 65], bf, tag="wv")
            wv1_s = res.tile([1, HPC * 65], bf, tag="wv1")
            bqk_s = res.tile([128, 4], fp32, tag="bqk")
            nc.sync.dma_start(bqk_s[:], bqk[:, :])
            nc.sync.dma_start(wv1_s[:], wvT[D:D + 1, :])
            for kc in range(NKC):
                nc.sync.dma_start(wq_s[:, kc * GD:(kc + 1) * GD],
                                  wqT[kc * 128:(kc + 1) * 128, :])
                nc.sync.dma_start(wk_s[:, kc * GD:(kc + 1) * GD],
                                  wkT[kc * 128:(kc + 1) * 128, :])
                nc.sync.dma_start(wv_s[:, kc * HPC * 65:(kc + 1) * HPC * 65],
                                  wvT[kc * 128:(kc + 1) * 128, :])
            if mode == "causal":
                cm_s = res.tile([128, 4 * QC], bf, tag="cm")
                for j in range(4):
                    nc.sync.dma_start(cm_s[:, j * QC:(j + 1) * QC],
                                      cmask[j, :, :])

            # resident projected activations
            qt_s = [res.tile([128, S], bf, tag=f"qt{m}", name=f"qt{m}")
                    for m in range(2)]
            kt_s = [res.tile([128, S], bf, tag=f"kt{m}", name=f"kt{m}")
                    for m in range(2)]
            vh_s = res.tile([128, N_KB * HPC * 65], bf, tag="vh")

            for sc in range(N_QC):
                cs = slice(sc * QC, (sc + 1) * QC)
                # ---- load x^T chunks for this seq chunk ----
                xq_t = []
                xk_t = []
                xv_t = []
                for kc in range(NKC):
                    t = xin.tile([128, QC], bf, tag="xq")
                    nc.sync.dma_start(t[:], xqT[kc * 128:(kc + 1) * 128, cs])
                    xq_t.append(t)
                    t = xin.tile([128, QC], bf, tag="xk")
                    nc.sync.dma_start(t[:], xkT[kc * 128:(kc + 1) * 128, cs])
                    xk_t.append(t)
                    t = xin.tile([128, QC], bf, tag="xv")
                    nc.sync.dma_start(t[:], xvT[kc * 128:(kc + 1) * 128, cs])
                    xv_t.append(t)
                xv1_t = xrow.tile([1, QC], bf, tag="xv1")
                nc.sync.dma_start(xv1_t[:], xvT[D:D + 1, cs])

                # ---- K and Q projections (transposed layout) ----
                for w_s, x_t, dst, bcol in ((wk_s, xk_t, kt_s, 2),
                                            (wq_s, xq_t, qt_s, 0)):
                    for m in range(2):
                        ps = pproj.tile([128, QC], fp32, tag="proj")
                        for kc in range(NKC):
                            nc.tensor.matmul(
                                ps[:],
                                w_s[:, kc * GD + m * 128: kc * GD + (m + 1) * 128],
                                x_t[kc][:],
                                start=(kc == 0), stop=(kc == NKC - 1))
                        nc.scalar.activation(dst[m][:, cs], ps[:], AF.Identity,
                                             bias=bqk_s[:, bcol + m:bcol + m + 1])

                # ---- V projection (natural layout, 65 cols/head) ----
                W65 = HPC * 65
                for j in range(QC // 128):
                    sb = sc * (QC // 128) + j
                    ps = pproj.tile([128, W65], fp32, tag="proj")
                    for kc in range(NKC):
                        nc.tensor.matmul(
                            ps[:],
                            xv_t[kc][:, j * 128:(j + 1) * 128],
                            wv_s[:, kc * W65:(kc + 1) * W65],
                            start=(kc == 0), stop=False)
                    nc.tensor.matmul(ps[:], xv1_t[:, j * 128:(j + 1) * 128],
                                     wv1_s[:], start=False, stop=True)
                    nc.vector.tensor_copy(vh_s[:, sb * W65:(sb + 1) * W65], ps[:])

                # ---- attention for q-chunk qc = sc ----
                n_kb = 4 * sc + 4 if mode == "causal" else N_KB
                for p in range(2):
                    pv = [ppv.tile([65, QC], fp32, tag="pv", name=f"pv{b2}")
                          for b2 in range(2)]
                    for kb in range(n_kb):
                        st = pscore.tile([128, 2 * QC], fp32, tag="s")
                        for b2 in range(2):
                            nc.tensor.matmul(
                                st[:, b2 * QC:(b2 + 1) * QC],
                                kt_s[p][b2 * 64:(b2 + 1) * 64,
                                        kb * 128:(kb + 1) * 128],
                                qt_s[p][b2 * 64:(b2 + 1) * 64, cs],
                                start=True, stop=True)
                        if mode == "causal":
                            j = kb - 4 * sc
                            if j >= 0:
                                for b2 in range(2):
                                    nc.vector.tensor_add(
                                        st[:, b2 * QC:(b2 + 1) * QC],
                                        st[:, b2 * QC:(b2 + 1) * QC],
                                        cm_s[:, j * QC:(j + 1) * QC])
                        elif mode == "general":
                            mt = mload.tile([128, QC], bf, tag="mt")
                            nc.sync.dma_start(
                                mt[:], amaskT[kb * 128:(kb + 1) * 128, cs])
                            for b2 in range(2):
                                nc.vector.tensor_add(
                                    st[:, b2 * QC:(b2 + 1) * QC],
                                    st[:, b2 * QC:(b2 + 1) * QC], mt[:])
                        et = eload.tile([128, 2 * QC], bf, tag="e")
                        nc.scalar.activation(et[:], st[:], AF.Exp)
                        for b2 in range(2):
                            h = 2 * p + b2
                            nc.tensor.matmul(
                                pv[b2][:],
                                vh_s[:, kb * W65 + h * 65: kb * W65 + h * 65 + 65],
                                et[:, b2 * QC:(b2 + 1) * QC],
                                start=(kb == 0), stop=(kb == n_kb - 1))
                    for b2 in range(2):
                        h = 2 * p + b2
                        ot = oout.tile([65, QC], fp32, tag="o")
                        nc.vector.tensor_copy(ot[:], pv[b2][:])
                        nc.sync.dma_start(out[h, :, cs], ot[:])

    nc.compile()
    return nc


def _get_nc(mode: str):
    if mode not in _cache:
        _cache[mode] = _build(mode)
    return _cache[mode]


def kernel(q, k, v, mask, Wq, bq, Wk, bk, Wv, bv):
    q = np.asarray(q, np.float32)
    k = np.asarray(k, np.float32)
    v = np.asarray(v, np.float32)
    Wq = np.asarray(Wq, np.float32)
    Wk = np.asarray(Wk, np.float32)
    Wv = np.asarray(Wv, np.float32)
    bq = np.asarray(bq, np.float32)
    bk = np.asarray(bk, np.float32)
    bv = np.asarray(bv, np.float32)
    m2 = np.asarray(mask)[0, 0]

    causal = bool(np.array_equal(m2 != 0, np.tril(np.ones((S, S), bool))))
    if causal:
        mode = "causal"
    elif np.all(m2 != 0):
        mode = "none"
    else:
        mode = "general"

    from concourse.bass_utils import run_bass_kernel_spmd

    nc = _get_nc(mode)

    ones_row = np.ones((1, S), np.float32)
    in_maps = []
    for c in range(N_CORES):
        b, g = divmod(c, HPC)
        gsl = slice(g * GD, (g + 1) * GD)
        # V weights: per head 65 cols (64 data + ones output-dim), plus a
        # bias row at the bottom (row D) multiplying the ones-row of x^T.
        wv65 = np.zeros((D + 1, HPC * 65), np.float32)
        for h in range(HPC):
            wv65[:D, h * 65:h * 65 + 64] = Wv[g * GD + h * DK:
                                              g * GD + h * DK + DK, :].T
            wv65[D, h * 65:h * 65 + 64] = bv[g * GD + h * DK:
                                             g * GD + h * DK + DK]
            wv65[D, h * 65 + 64] = 1.0
        im = {
            "xqT": q[b].T.astype(bf16),
            "xkT": k[b].T.astype(bf16),
            "xvT": np.concatenate([v[b].T, ones_row], 0).astype(bf16),
            "wqT": (Wq[gsl, :].T / 8.0).astype(bf16),
            "wkT": Wk[gsl, :].T.astype(bf16),
            "wvT": wv65.astype(bf16),
            "bqk": np.stack([bq[gsl][:128] / 8.0, bq[gsl][128:] / 8.0,
                             bk[gsl][:128], bk[gsl][128:]], 1)
                     .astype(np.float32).copy(),
        }
        if mode == "causal":
            # variant j for block (qc, kb=4qc+j): keep iff r <= c - 128j
            r = np.arange(128)[:, None]
            cc = np.arange(QC)[None, :]
            cm = np.stack([np.where(r <= cc - 128 * j, 0.0, -1e9)
                           for j in range(4)]).astype(bf16)
            im["cmask"] = cm
        elif mode == "general":
            add = np.where(m2 == 0, -1e9, 0.0).astype(np.float32)
            im["amaskT"] = add.T.astype(bf16).copy()
        in_maps.append(im)

    global _last_in_maps
    _last_in_maps = in_maps
    res = run_bass_kernel_spmd(nc, in_maps, core_ids=list(range(N_CORES)))

    outf = np.empty((B, S, D), np.float32)
    for c in range(N_CORES):
        b, g = divmod(c, HPC)
        o = res.results[c]["out"]  # [HPC, 65, S]
        num = o[:, :64, :]         # [HPC, 64, S]
        den = o[:, 64:65, :]       # [HPC, 1, S]
        oh = num / den             # [HPC, 64, S]
        outf[b, :, g * GD:(g + 1) * GD] = (
            oh.transpose(2, 0, 1).reshape(S, GD))
    return outf


# revision 5
# speedup vs baseline: 1.1499x; 1.1499x over previous
"""Multi-head attention (B=2, S=2048, D=1024, H=16, Dk=64) on 8 TRN2 cores.

Sharding: core c handles batch b=c//4 and head group g=c%4 (heads 4g..4g+3,
i.e. projection output dims 256g..256g+256). Fully independent cores, no
collectives.

Device pipeline per core (all matmul inputs bf16, fp32 PSUM accumulation):
  - K/Q projections into transposed layout  QT/KT [256 dims, 2048 seq]
    (lhsT = W^T chunk, rhs = x^T chunk; per-partition bias added on DVE
    during the PSUM->SBUF copy; Wq pre-scaled by 1/8 = 1/sqrt(Dk) on host).
  - V projection into natural layout VH [seq, dims], with host-augmented
    weights: per head 65 columns where col 64 is an all-ones output dim
    (gives the softmax denominator for free), and an extra ones-row in x^T
    + bias-row in W^T implements the bias add (K=1 extra accumulation).
  - Scores computed transposed: S^T[kv,q] = KT^T-block @ QT-block, two heads
    row-packed (concurrent 64-row strips) into one [128,1024] PSUM tile.
    Causal handling: upper blocks skipped; diagonal blocks get a [128,128]
    tril-window mask add on DVE, fully-masked column ranges are skipped by
    the exp AP and zeroed in E by GpSimd memsets. No max-subtraction
    (|scores| <= ~4). E in bf16.
  - PV: O'^T[65, q] += VH'-block (as stationary) @ E-block, accumulated over
    kv blocks in PSUM. Row 64 = sum(E) = softmax denominator.
  - O'^T copied to SBUF (DVE) and DMA'd out (GpSimd queues); final division
    + head interleave on host.
"""

import numpy as np
import ml_dtypes

B, S, D, H, DK = 2, 2048, 1024, 16, 64
N_CORES = 8
HPC = 4          # heads per core
GD = HPC * DK    # group dims = 256
W65 = HPC * 65   # V-projection output cols (64 data + 1 ones per head)
QC = 512         # q-chunk (also seq projection chunk)
N_QC = S // QC   # 4
N_KB = S // 128  # 16
NKC = D // 128   # 8 contraction chunks
bf16 = ml_dtypes.bfloat16

_cache: dict = {}


def _build(mode: str):
    """mode: 'causal' (diag-window masks, upper blocks skipped),
    'none' (no masking, all blocks), 'general' (per-block masks from DRAM)."""
    import concourse.bass as bass
    import concourse.mybir as mybir
    from concourse import bacc
    from concourse.tile import TileContext

    fp32 = mybir.dt.float32
    bf = mybir.dt.bfloat16
    AF = mybir.ActivationFunctionType

    nc = bacc.Bacc("TRN2", target_bir_lowering=False, debug=False,
                   num_devices=N_CORES)

    # host-prepacked inputs (see kernel() below)
    xq = nc.dram_tensor("xq", [NKC, 128, S], bf, kind="ExternalInput")
    xk = nc.dram_tensor("xk", [NKC, 128, S], bf, kind="ExternalInput")
    xv = nc.dram_tensor("xv", [NKC, 128, S], bf, kind="ExternalInput")
    xv1 = nc.dram_tensor("xv1", [1, S], bf, kind="ExternalInput")
    wq = nc.dram_tensor("wq", [128, NKC * GD], bf, kind="ExternalInput")
    wk = nc.dram_tensor("wk", [128, NKC * GD], bf, kind="ExternalInput")
    wv = nc.dram_tensor("wv", [128, NKC * W65], bf, kind="ExternalInput")
    wv1 = nc.dram_tensor("wv1", [1, W65], bf, kind="ExternalInput")
    bqk = nc.dram_tensor("bqk", [128, 4], fp32, kind="ExternalInput")
    if mode == "causal":
        cmw = nc.dram_tensor("cmw", [128, 128], bf, kind="ExternalInput")
    elif mode == "general":
        amaskT = nc.dram_tensor("amaskT", [S, S], bf, kind="ExternalInput")
    out = nc.dram_tensor("out", [HPC, 65, S], fp32, kind="ExternalOutput")

    with TileContext(nc) as tc:
        with (
            tc.tile_pool(name="res", bufs=1) as res,
            tc.tile_pool(name="mload", bufs=4) as mload,
            tc.tile_pool(name="eload", bufs=4) as eload,
            tc.tile_pool(name="oout", bufs=4) as oout,
            tc.tile_pool(name="pproj", bufs=2, space="PSUM") as pproj,
            tc.tile_pool(name="pscore", bufs=2, space="PSUM") as pscore,
            tc.tile_pool(name="ppv", bufs=2, space="PSUM") as ppv,
        ):
            # ---- resident loads: X first (K input unblocks PE first) ----
            xk_s = res.tile([128, NKC * S], bf, tag="xk")
            xq_s = res.tile([128, NKC * S], bf, tag="xq")
            xv_s = res.tile([128, NKC * S], bf, tag="xv")
            wq_s = res.tile([128, NKC * GD], bf, tag="wq")
            wk_s = res.tile([128, NKC * GD], bf, tag="wk")
            wv_s = res.tile([128, NKC * W65], bf, tag="wv")
            wv1_s = res.tile([1, W65], bf, tag="wv1")
            xv1_s = res.tile([1, S], bf, tag="xv1")
            bqk_s = res.tile([128, 4], fp32, tag="bqk")

            nc.sync.dma_start(wk_s[:], wk[:, :])
            nc.sync.dma_start(bqk_s[:], bqk[:, :])
            for kc in range(NKC):
                nc.sync.dma_start(xk_s[:, kc * S:(kc + 1) * S], xk[kc, :, :])
            nc.sync.dma_start(wq_s[:], wq[:, :])
            for kc in range(NKC):
                nc.sync.dma_start(xq_s[:, kc * S:(kc + 1) * S], xq[kc, :, :])
            nc.sync.dma_start(wv_s[:], wv[:, :])
            nc.sync.dma_start(wv1_s[:], wv1[:, :])
            nc.sync.dma_start(xv1_s[:], xv1[:, :])
            for kc in range(NKC):
                nc.sync.dma_start(xv_s[:, kc * S:(kc + 1) * S], xv[kc, :, :])
            if mode == "causal":
                cmw_s = res.tile([128, 128], bf, tag="cmw")
                nc.gpsimd.dma_start(cmw_s[:], cmw[:, :])

            # resident projected activations
            qt_s = [res.tile([128, S], bf, tag=f"qt{m}", name=f"qt{m}")
                    for m in range(2)]
            kt_s = [res.tile([128, S], bf, tag=f"kt{m}", name=f"kt{m}")
                    for m in range(2)]
            vh_s = res.tile([128, N_KB * W65], bf, tag="vh")

            for sc in range(N_QC):
                cs = slice(sc * QC, (sc + 1) * QC)
                # ---- K and Q projections (transposed layout) ----
                for w_s, x_s, dst, bcol in ((wk_s, xk_s, kt_s, 2),
                                            (wq_s, xq_s, qt_s, 0)):
                    for m in range(2):
                        ps = pproj.tile([128, QC], fp32, tag="proj")
                        for kc in range(NKC):
                            nc.tensor.matmul(
                                ps[:],
                                w_s[:, kc * GD + m * 128: kc * GD + (m + 1) * 128],
                                x_s[:, kc * S + sc * QC: kc * S + (sc + 1) * QC],
                                start=(kc == 0), stop=(kc == NKC - 1))
                        nc.vector.tensor_scalar_add(
                            dst[m][:, cs], ps[:],
                            bqk_s[:, bcol + m:bcol + m + 1])

                # ---- V projection (natural layout, 65 cols/head) ----
                for j in range(QC // 128):
                    sb = sc * (QC // 128) + j
                    so = sc * QC + j * 128
                    ps = pproj.tile([128, W65], fp32, tag="proj")
                    for kc in range(NKC):
                        nc.tensor.matmul(
                            ps[:],
                            xv_s[:, kc * S + so: kc * S + so + 128],
                            wv_s[:, kc * W65:(kc + 1) * W65],
                            start=(kc == 0), stop=False)
                    nc.tensor.matmul(ps[:], xv1_s[:, so:so + 128],
                                     wv1_s[:], start=False, stop=True)
                    nc.vector.tensor_copy(vh_s[:, sb * W65:(sb + 1) * W65],
                                          ps[:])

                # ---- attention for q-chunk qc = sc ----
                n_kb = 4 * sc + 4 if mode == "causal" else N_KB
                for p in range(2):
                    pv = [ppv.tile([65, QC], fp32, tag="pv", name=f"pv{b2}")
                          for b2 in range(2)]
                    for kb in range(n_kb):
                        st = pscore.tile([128, 2 * QC], fp32, tag="s")
                        for b2 in range(2):
                            nc.tensor.matmul(
                                st[:, b2 * QC:(b2 + 1) * QC],
                                kt_s[p][b2 * 64:(b2 + 1) * 64,
                                        kb * 128:(kb + 1) * 128],
                                qt_s[p][b2 * 64:(b2 + 1) * 64, cs],
                                start=True, stop=True)
                        et = eload.tile([128, 2 * QC], bf, tag="e")
                        j = kb - 4 * sc if mode == "causal" else -1
                        if j >= 0:
                            # diagonal block: mask window + trimmed exp
                            for b2 in range(2):
                                nc.vector.tensor_add(
                                    st[:, b2 * QC + j * 128:
                                           b2 * QC + (j + 1) * 128],
                                    st[:, b2 * QC + j * 128:
                                           b2 * QC + (j + 1) * 128],
                                    cmw_s[:])
                            if j > 0:
                                for b2 in range(2):
                                    nc.gpsimd.memset(
                                        et[:, b2 * QC: b2 * QC + j * 128], 0.0)
                                st3 = st[:].rearrange("p (h n) -> p h n", h=2)
                                et3 = et[:].rearrange("p (h n) -> p h n", h=2)
                                nc.scalar.activation(
                                    et3[:, :, j * 128:], st3[:, :, j * 128:],
                                    AF.Exp)
                            else:
                                nc.scalar.activation(et[:], st[:], AF.Exp)
                        else:
                            if mode == "general":
                                mt = mload.tile([128, QC], bf, tag="mt")
                                nc.sync.dma_start(
                                    mt[:], amaskT[kb * 128:(kb + 1) * 128, cs])
                                for b2 in range(2):
                                    nc.vector.tensor_add(
                                        st[:, b2 * QC:(b2 + 1) * QC],
                                        st[:, b2 * QC:(b2 + 1) * QC], mt[:])
                            nc.scalar.activation(et[:], st[:], AF.Exp)
                        for b2 in range(2):
                            h = 2 * p + b2
                            nc.tensor.matmul(
                                pv[b2][:],
                                vh_s[:, kb * W65 + h * 65:
                                        kb * W65 + h * 65 + 65],
                                et[:, b2 * QC:(b2 + 1) * QC],
                                start=(kb == 0), stop=(kb == n_kb - 1))
                    for b2 in range(2):
                        h = 2 * p + b2
                        ot = oout.tile([65, QC], fp32, tag="o")
                        nc.vector.tensor_copy(ot[:], pv[b2][:])
                        nc.gpsimd.dma_start(out[h, :, cs], ot[:])

    nc.compile()
    return nc


def _get_nc(mode: str):
    if mode not in _cache:
        _cache[mode] = _build(mode)
    return _cache[mode]


def kernel(q, k, v, mask, Wq, bq, Wk, bk, Wv, bv):
    q = np.asarray(q, np.float32)
    k = np.asarray(k, np.float32)
    v = np.asarray(v, np.float32)
    Wq = np.asarray(Wq, np.float32)
    Wk = np.asarray(Wk, np.float32)
    Wv = np.asarray(Wv, np.float32)
    bq = np.asarray(bq, np.float32)
    bk = np.asarray(bk, np.float32)
    bv = np.asarray(bv, np.float32)
    m2 = np.asarray(mask)[0, 0]

    causal = bool(np.array_equal(m2 != 0, np.tril(np.ones((S, S), bool))))
    if causal:
        mode = "causal"
    elif np.all(m2 != 0):
        mode = "none"
    else:
        mode = "general"

    from concourse.bass_utils import run_bass_kernel_spmd

    nc = _get_nc(mode)

    in_maps = []
    for c in range(N_CORES):
        b, g = divmod(c, HPC)
        gsl = slice(g * GD, (g + 1) * GD)
        # V weights: per head 65 cols (64 data + ones output-dim), plus a
        # bias row at the bottom (row D) multiplying the ones-row of x^T.
        wv65 = np.zeros((D + 1, W65), np.float32)
        for h in range(HPC):
            wv65[:D, h * 65:h * 65 + 64] = Wv[g * GD + h * DK:
                                              g * GD + h * DK + DK, :].T
            wv65[D, h * 65:h * 65 + 64] = bv[g * GD + h * DK:
                                             g * GD + h * DK + DK]
            wv65[D, h * 65 + 64] = 1.0
        # pack W^T [D, n] -> [128, NKC*n] (chunk kc at cols kc*n..)
        def packw(wt):
            n = wt.shape[1]
            return np.ascontiguousarray(
                wt[:D].reshape(NKC, 128, n).transpose(1, 0, 2).reshape(128, NKC * n)
            ).astype(bf16)

        im = {
            "xq": np.ascontiguousarray(q[b].T.reshape(NKC, 128, S)).astype(bf16),
            "xk": np.ascontiguousarray(k[b].T.reshape(NKC, 128, S)).astype(bf16),
            "xv": np.ascontiguousarray(v[b].T.reshape(NKC, 128, S)).astype(bf16),
            "xv1": np.ones((1, S), bf16),
            "wq": packw(Wq[gsl, :].T / 8.0),
            "wk": packw(Wk[gsl, :].T),
            "wv": packw(wv65),
            "wv1": wv65[D:D + 1, :].astype(bf16).copy(),
            "bqk": np.stack([bq[gsl][:128] / 8.0, bq[gsl][128:] / 8.0,
                             bk[gsl][:128], bk[gsl][128:]], 1)
                     .astype(np.float32).copy(),
        }
        if mode == "causal":
            r = np.arange(128)[:, None]
            cc = np.arange(128)[None, :]
            im["cmw"] = np.where(r <= cc, 0.0, -1e9).astype(bf16)
        elif mode == "general":
            add = np.where(m2 == 0, -1e9, 0.0).astype(np.float32)
            im["amaskT"] = add.T.astype(bf16).copy()
        in_maps.append(im)

    global _last_in_maps
    _last_in_maps = in_maps
    res = run_bass_kernel_spmd(nc, in_maps, core_ids=list(range(N_CORES)))

    outf = np.empty((B, S, D), np.float32)
    for c in range(N_CORES):
        b, g = divmod(c, HPC)
        o = res.results[c]["out"]  # [HPC, 65, S]
        num = o[:, :64, :]         # [HPC, 64, S]
        den = o[:, 64:65, :]       # [HPC, 1, S]
        oh = num / den             # [HPC, 64, S]
        outf[b, :, g * GD:(g + 1) * GD] = (
            oh.transpose(2, 0, 1).reshape(S, GD))
    return outf


# revision 6
# speedup vs baseline: 1.2411x; 1.0793x over previous
"""Multi-head attention (B=2, S=2048, D=1024, H=16, Dk=64) on 8 TRN2 cores.

Sharding: core c handles batch b=c//4 and head group g=c%4 (heads 4g..4g+3,
i.e. projection output dims 256g..256g+256). Fully independent cores, no
collectives.

Device pipeline per core (all matmul inputs bf16, fp32 PSUM accumulation):
  - K/Q projections into transposed layout  QT/KT [256 dims, 2048 seq]
    (lhsT = W^T chunk, rhs = x^T chunk; two seq-chunks per weight load to
    amortize LDWEIGHTS; per-partition bias added on DVE during the
    PSUM->SBUF copy; Wq pre-scaled by 1/8 = 1/sqrt(Dk) on host).
  - V projection into natural layout VH [seq, dims] with per-head 65 cols
    (col 64 is an all-ones output dim giving the softmax denominator);
    bias + the ones column added via a broadcast tile in the DVE copy.
  - Scores computed transposed: S^T[kv,q] = KT-block (stationary) @ QT-chunk,
    a head pair sharing one [128,1024] PSUM tile. Causal: upper blocks
    skipped; diagonal blocks N-trimmed in the matmul, masked with a
    [128,128] tril-window add on DVE, exp AP trimmed to match. No
    max-subtraction (|scores| <= ~4). E in bf16.
  - PV: O'^T[65, q] += VH'-block (stationary) @ E-block, accumulated over
    kv blocks in PSUM, N-trimmed on diagonal blocks. Row 64 = sum(E).
  - O'^T copied to SBUF (DVE) and DMA'd out (GpSimd queues); final division
    + head interleave on host.
"""

import numpy as np
import ml_dtypes

B, S, D, H, DK = 2, 2048, 1024, 16, 64
N_CORES = 8
HPC = 4          # heads per core
GD = HPC * DK    # group dims = 256
W65 = HPC * 65   # V-projection output cols (64 data + 1 ones per head)
QC = 512         # q-chunk (also seq projection chunk)
N_QC = S // QC   # 4
N_KB = S // 128  # 16
NKC = D // 128   # 8 contraction chunks
bf16 = ml_dtypes.bfloat16

_cache: dict = {}


def _build(mode: str):
    """mode: 'causal' (diag-window masks, upper blocks skipped),
    'none' (no masking, all blocks), 'general' (per-block masks from DRAM)."""
    import concourse.bass as bass
    import concourse.mybir as mybir
    from concourse import bacc
    from concourse.tile import TileContext

    fp32 = mybir.dt.float32
    bf = mybir.dt.bfloat16
    AF = mybir.ActivationFunctionType

    nc = bacc.Bacc("TRN2", target_bir_lowering=False, debug=False,
                   num_devices=N_CORES)

    # host-prepacked inputs (see kernel() below)
    xq = nc.dram_tensor("xq", [NKC, 128, S], bf, kind="ExternalInput")
    xk = nc.dram_tensor("xk", [NKC, 128, S], bf, kind="ExternalInput")
    xv = nc.dram_tensor("xv", [NKC, 128, S], bf, kind="ExternalInput")
    wq = nc.dram_tensor("wq", [128, NKC * GD], bf, kind="ExternalInput")
    wk = nc.dram_tensor("wk", [128, NKC * GD], bf, kind="ExternalInput")
    wv = nc.dram_tensor("wv", [128, NKC * W65], bf, kind="ExternalInput")
    vb = nc.dram_tensor("vb", [128, W65], bf, kind="ExternalInput")
    bqk = nc.dram_tensor("bqk", [128, 4], fp32, kind="ExternalInput")
    if mode == "causal":
        cmw = nc.dram_tensor("cmw", [128, 128], bf, kind="ExternalInput")
    elif mode == "general":
        amaskT = nc.dram_tensor("amaskT", [S, S], bf, kind="ExternalInput")
    out = nc.dram_tensor("out", [HPC, 65, S], fp32, kind="ExternalOutput")

    HS = S // 2  # DMA half

    with TileContext(nc) as tc:
        with (
            tc.tile_pool(name="res", bufs=1) as res,
            tc.tile_pool(name="mload", bufs=4) as mload,
            tc.tile_pool(name="eload", bufs=4) as eload,
            tc.tile_pool(name="oout", bufs=4) as oout,
            tc.tile_pool(name="pproj", bufs=2, space="PSUM") as pproj,
            tc.tile_pool(name="pscore", bufs=2, space="PSUM") as pscore,
            tc.tile_pool(name="ppv", bufs=2, space="PSUM") as ppv,
        ):
            # ---- resident loads: K-path first so PE unblocks fastest ----
            xk_s = res.tile([128, NKC * S], bf, tag="xk")
            xq_s = res.tile([128, NKC * S], bf, tag="xq")
            xv_s = res.tile([128, NKC * S], bf, tag="xv")
            wq_s = res.tile([128, NKC * GD], bf, tag="wq")
            wk_s = res.tile([128, NKC * GD], bf, tag="wk")
            wv_s = res.tile([128, NKC * W65], bf, tag="wv")
            vb_s = res.tile([128, W65], bf, tag="vb")
            bqk_s = res.tile([128, 4], fp32, tag="bqk")

            def xhalf(dst, src, kc, h):
                nc.sync.dma_start(
                    dst[:, kc * S + h * HS: kc * S + (h + 1) * HS],
                    src[kc, :, h * HS:(h + 1) * HS])

            nc.sync.dma_start(wk_s[:], wk[:, :])
            nc.sync.dma_start(bqk_s[:], bqk[:, :])
            for kc in range(NKC):
                xhalf(xk_s, xk, kc, 0)
            nc.sync.dma_start(wq_s[:], wq[:, :])
            for kc in range(NKC):
                xhalf(xq_s, xq, kc, 0)
            nc.sync.dma_start(wv_s[:], wv[:, :])
            nc.gpsimd.dma_start(vb_s[:], vb[:, :])
            for kc in range(NKC):
                xhalf(xv_s, xv, kc, 0)
            for kc in range(NKC):
                xhalf(xk_s, xk, kc, 1)
            for kc in range(NKC):
                xhalf(xq_s, xq, kc, 1)
            for kc in range(NKC):
                xhalf(xv_s, xv, kc, 1)
            if mode == "causal":
                cmw_s = res.tile([128, 128], bf, tag="cmw")
                nc.gpsimd.dma_start(cmw_s[:], cmw[:, :])

            # resident projected activations
            qt_s = [res.tile([128, S], bf, tag=f"qt{m}", name=f"qt{m}")
                    for m in range(2)]
            kt_s = [res.tile([128, S], bf, tag=f"kt{m}", name=f"kt{m}")
                    for m in range(2)]
            vh_s = res.tile([128, N_KB * W65], bf, tag="vh")

            def attention(sc):
                cs = slice(sc * QC, (sc + 1) * QC)
                n_kb = 4 * sc + 4 if mode == "causal" else N_KB
                for p in range(2):
                    pv = [ppv.tile([65, QC], fp32, tag="pv", name=f"pv{b2}")
                          for b2 in range(2)]
                    for kb in range(n_kb):
                        j = kb - 4 * sc if mode == "causal" else -1
                        t = 128 * j if j > 0 else 0  # trimmed leading cols
                        st = pscore.tile([128, 2 * QC], fp32, tag="s")
                        for b2 in range(2):
                            nc.tensor.matmul(
                                st[:, b2 * QC + t:(b2 + 1) * QC],
                                kt_s[p][b2 * 64:(b2 + 1) * 64,
                                        kb * 128:(kb + 1) * 128],
                                qt_s[p][b2 * 64:(b2 + 1) * 64,
                                        sc * QC + t:(sc + 1) * QC],
                                start=True, stop=True)
                        et = eload.tile([128, 2 * QC], bf, tag="e")
                        if j >= 0:
                            for b2 in range(2):
                                nc.vector.tensor_add(
                                    st[:, b2 * QC + t: b2 * QC + t + 128],
                                    st[:, b2 * QC + t: b2 * QC + t + 128],
                                    cmw_s[:])
                        if t > 0:
                            st3 = st[:].rearrange("p (h n) -> p h n", h=2)
                            et3 = et[:].rearrange("p (h n) -> p h n", h=2)
                            nc.scalar.activation(et3[:, :, t:], st3[:, :, t:],
                                                 AF.Exp)
                        else:
                            if mode == "general":
                                mt = mload.tile([128, QC], bf, tag="mt")
                                nc.sync.dma_start(
                                    mt[:], amaskT[kb * 128:(kb + 1) * 128, cs])
                                for b2 in range(2):
                                    nc.vector.tensor_add(
                                        st[:, b2 * QC:(b2 + 1) * QC],
                                        st[:, b2 * QC:(b2 + 1) * QC], mt[:])
                            nc.scalar.activation(et[:], st[:], AF.Exp)
                        for b2 in range(2):
                            h = 2 * p + b2
                            nc.tensor.matmul(
                                pv[b2][:, t:],
                                vh_s[:, kb * W65 + h * 65:
                                        kb * W65 + h * 65 + 65],
                                et[:, b2 * QC + t:(b2 + 1) * QC],
                                start=(kb == 0), stop=(kb == n_kb - 1))
                    for b2 in range(2):
                        h = 2 * p + b2
                        ot = oout.tile([65, QC], fp32, tag="o")
                        nc.vector.tensor_copy(ot[:], pv[b2][:])
                        nc.gpsimd.dma_start(out[h, :, cs], ot[:])

            for scp in range(N_QC // 2):
                sc0, sc1 = 2 * scp, 2 * scp + 1
                # ---- K and Q projections, two seq-chunks per weight load ----
                for w_s, x_s, dst, bcol in ((wk_s, xk_s, kt_s, 2),
                                            (wq_s, xq_s, qt_s, 0)):
                    for m in range(2):
                        psA = pproj.tile([128, QC], fp32, tag="proj",
                                         name="psA")
                        psB = pproj.tile([128, QC], fp32, tag="proj",
                                         name="psB")
                        for kc in range(NKC):
                            wsl = w_s[:, kc * GD + m * 128:
                                      kc * GD + (m + 1) * 128]
                            nc.tensor.matmul(
                                psA[:], wsl,
                                x_s[:, kc * S + sc0 * QC:
                                       kc * S + (sc0 + 1) * QC],
                                start=(kc == 0), stop=(kc == NKC - 1))
                            nc.tensor.matmul(
                                psB[:], wsl,
                                x_s[:, kc * S + sc1 * QC:
                                       kc * S + (sc1 + 1) * QC],
                                start=(kc == 0), stop=(kc == NKC - 1))
                        bias = bqk_s[:, bcol + m:bcol + m + 1]
                        nc.vector.tensor_scalar_add(
                            dst[m][:, sc0 * QC:(sc0 + 1) * QC], psA[:], bias)
                        nc.vector.tensor_scalar_add(
                            dst[m][:, sc1 * QC:(sc1 + 1) * QC], psB[:], bias)

                # ---- V projection (natural layout, 65 cols/head) ----
                for sb in range(sc0 * 4, (sc1 + 1) * 4):
                    so = sb * 128
                    ps = pproj.tile([128, W65], fp32, tag="proj")
                    for kc in range(NKC):
                        nc.tensor.matmul(
                            ps[:],
                            xv_s[:, kc * S + so: kc * S + so + 128],
                            wv_s[:, kc * W65:(kc + 1) * W65],
                            start=(kc == 0), stop=(kc == NKC - 1))
                    nc.vector.tensor_add(vh_s[:, sb * W65:(sb + 1) * W65],
                                         ps[:], vb_s[:])

                attention(sc0)
                attention(sc1)

    nc.compile()
    return nc


def _get_nc(mode: str):
    if mode not in _cache:
        _cache[mode] = _build(mode)
    return _cache[mode]


def kernel(q, k, v, mask, Wq, bq, Wk, bk, Wv, bv):
    q = np.asarray(q, np.float32)
    k = np.asarray(k, np.float32)
    v = np.asarray(v, np.float32)
    Wq = np.asarray(Wq, np.float32)
    Wk = np.asarray(Wk, np.float32)
    Wv = np.asarray(Wv, np.float32)
    bq = np.asarray(bq, np.float32)
    bk = np.asarray(bk, np.float32)
    bv = np.asarray(bv, np.float32)
    m2 = np.asarray(mask)[0, 0]

    causal = bool(np.array_equal(m2 != 0, np.tril(np.ones((S, S), bool))))
    if causal:
        mode = "causal"
    elif np.all(m2 != 0):
        mode = "none"
    else:
        mode = "general"

    from concourse.bass_utils import run_bass_kernel_spmd

    nc = _get_nc(mode)

    in_maps = []
    for c in range(N_CORES):
        b, g = divmod(c, HPC)
        gsl = slice(g * GD, (g + 1) * GD)
        # V weights: per head 65 cols (64 data + zero col for the ones dim);
        # the ones + bias come from the broadcast add tile vb.
        wv65 = np.zeros((D, W65), np.float32)
        vbrow = np.zeros((1, W65), np.float32)
        for h in range(HPC):
            wv65[:, h * 65:h * 65 + 64] = Wv[g * GD + h * DK:
                                             g * GD + h * DK + DK, :].T
            vbrow[0, h * 65:h * 65 + 64] = bv[g * GD + h * DK:
                                              g * GD + h * DK + DK]
            vbrow[0, h * 65 + 64] = 1.0

        def packw(wt):
            n = wt.shape[1]
            return np.ascontiguousarray(
                wt.reshape(NKC, 128, n).transpose(1, 0, 2).reshape(128, NKC * n)
            ).astype(bf16)

        im = {
            "xq": np.ascontiguousarray(q[b].T.reshape(NKC, 128, S)).astype(bf16),
            "xk": np.ascontiguousarray(k[b].T.reshape(NKC, 128, S)).astype(bf16),
            "xv": np.ascontiguousarray(v[b].T.reshape(NKC, 128, S)).astype(bf16),
            "wq": packw(Wq[gsl, :].T / 8.0),
            "wk": packw(Wk[gsl, :].T),
            "wv": packw(wv65),
            "vb": np.broadcast_to(vbrow, (128, W65)).astype(bf16).copy(),
            "bqk": np.stack([bq[gsl][:128] / 8.0, bq[gsl][128:] / 8.0,
                             bk[gsl][:128], bk[gsl][128:]], 1)
                     .astype(np.float32).copy(),
        }
        if mode == "causal":
            r = np.arange(128)[:, None]
            cc = np.arange(128)[None, :]
            im["cmw"] = np.where(r <= cc, 0.0, -1e9).astype(bf16)
        elif mode == "general":
            add = np.where(m2 == 0, -1e9, 0.0).astype(np.float32)
            im["amaskT"] = add.T.astype(bf16).copy()
        in_maps.append(im)

    global _last_in_maps
    _last_in_maps = in_maps
    res = run_bass_kernel_spmd(nc, in_maps, core_ids=list(range(N_CORES)))

    outf = np.empty((B, S, D), np.float32)
    for c in range(N_CORES):
        b, g = divmod(c, HPC)
        o = res.results[c]["out"]  # [HPC, 65, S]
        num = o[:, :64, :]         # [HPC, 64, S]
        den = o[:, 64:65, :]       # [HPC, 1, S]
        oh = num / den             # [HPC, 64, S]
        outf[b, :, g * GD:(g + 1) * GD] = (
            oh.transpose(2, 0, 1).reshape(S, GD))
    return outf
